# revision 1
# baseline (speedup 1.0000x reference)
"""Multi-head causal attention (B=4, S=2048, H=16, hd=64) on 8 TRN2 NeuronCores.

Sharding: core c handles batch b = c//2 and head-group g = c%2 (8 heads each,
i.e. columns g*512:(g+1)*512 of the 1024-wide vec dim). Each core computes its
heads' attention context and a partial output projection against its 512
columns of w_out; the host sums the two partials per batch (the row-parallel
all-reduce done on host, since the contract is full-I/O anyway).

Per-core kernel layout (everything transposed: [feature, seq] on partitions):
  xT[512, 2048] -> per head-pair x2hT[128, 2048]
  qT/kT = blockdiag(W^T) @ x2hT            [128, 2048]  (2 heads packed)
  v     = x2hT_tile^T @ blockdiag(Wv^T)    [128 s, 128] per s-tile, stored as
          v_aug[128, 2, 16, 65] with a ones column -> row 64 of ctx psum
          accumulates the softmax denominator for free
  scoresT[k-tile 128, QB] = kT_tile^T @ qT_block   (causal: only k <= diag)
  E = exp(scores/8)   (no max-subtraction: inputs are N(0,1), |s| < ~7)
  ctxT[65, QB] += v_aug_tile^T @ E_tile   over k-tiles
  ctxT /= ctxT[64]    (reciprocal + gpsimd partition-broadcast + DVE mul)
  out[s-tile, 1024] = sum_p ctx_chunk_p^T @ woT_chunk_p
Matmuls run in float32r (TF32-like, 1 cycle/row at N>=256).
"""
import os
import sys

for _p in ("/opt/trn_rl_repo", "/root/.axon_site/_ro/trn_rl_repo"):
    if os.path.isdir(_p) and _p not in sys.path:
        sys.path.append(_p)

import numpy as np

B, S, V = 4, 2048, 1024
NH, HD = 16, 64
HPC = 8  # heads per core
PAIRS = HPC // 2
QB = 256  # query block (matmul moving dim)
NQB = S // QB
KT = 128  # key tile
DIAG = QB // KT  # diagonal k-tiles per q-block
GROUP = 1024 // QB  # k-tiles per exp group ([128, 1024] PSUM staging)
NEG = -1.0e30


def _build_nc():
    import concourse.bacc as bacc
    import concourse.tile as tile
    from concourse import mybir

    F32 = mybir.dt.float32
    F32R = mybir.dt.float32r
    EXPF = mybir.ActivationFunctionType.Exp

    nc = bacc.Bacc(None, target_bir_lowering=False)

    xT = nc.dram_tensor("xT", [PAIRS * 128, S], F32R, kind="ExternalInput")
    wq = nc.dram_tensor("wq", [128, 128], F32R, kind="ExternalInput")
    wk = nc.dram_tensor("wk", [128, 128], F32R, kind="ExternalInput")
    wv = nc.dram_tensor("wv", [128, 128], F32R, kind="ExternalInput")
    woT = nc.dram_tensor("woT", [PAIRS * 128, V], F32R, kind="ExternalInput")
    maskadd = nc.dram_tensor("maskadd", [DIAG, 128, QB], F32, kind="ExternalInput")
    vones = nc.dram_tensor("vones", [128, 2, S // KT, 1], F32R, kind="ExternalInput")
    out = nc.dram_tensor("out", [S, V], F32R, kind="ExternalOutput")

    with tile.TileContext(nc) as tc:
        with (
            tc.tile_pool(name="persist", bufs=1) as persist,
            tc.tile_pool(name="xstage", bufs=2) as xstage,
            tc.tile_pool(name="epool", bufs=6) as epool,
            tc.tile_pool(name="strip", bufs=2) as strip,
            tc.tile_pool(name="small", bufs=4) as small,
            tc.tile_pool(name="outsb", bufs=2) as outsb,
            tc.tile_pool(name="ps_s", bufs=2, space="PSUM") as ps_s,
            tc.tile_pool(name="ps_c", bufs=2, space="PSUM") as ps_c,
            tc.tile_pool(name="ps_o", bufs=2, space="PSUM") as ps_o,
        ):
            # ---- constants ----
            wq_sb = persist.tile([128, 128], F32R, tag="wq")
            wk_sb = persist.tile([128, 128], F32R, tag="wk")
            wv_sb = persist.tile([128, 128], F32R, tag="wv")
            nc.sync.dma_start(wq_sb[:], wq[:])
            nc.sync.dma_start(wk_sb[:], wk[:])
            nc.sync.dma_start(wv_sb[:], wv[:])
            woT_sb = persist.tile([128, PAIRS, V], F32R, tag="wo")
            for p in range(PAIRS):
                nc.sync.dma_start(woT_sb[:, p, :], woT[p * 128 : (p + 1) * 128, :])
            mask_sb = persist.tile([128, DIAG, QB], F32, tag="mask")
            for t in range(DIAG):
                nc.sync.dma_start(mask_sb[:, t, :], maskadd[t])

            # ---- phase 1: QKV for all head pairs ----
            qT = []  # per pair [128, S]
            kT = []
            vaug = []  # per pair [128, 2, S // KT, 65]
            for p in range(PAIRS):
                x2 = xstage.tile([128, S], F32R, tag="x2")
                nc.sync.dma_start(x2[:], xT[p * 128 : (p + 1) * 128, :])

                q_sb = persist.tile([128, S], F32R, tag=f"q{p}")
                k_sb = persist.tile([128, S], F32R, tag=f"k{p}")
                for w_sb, dst in ((wq_sb, q_sb), (wk_sb, k_sb)):
                    for h in range(2):  # halves of S, 1024 each
                        pq = ps_s.tile([128, 1024], F32, tag="s_ps")
                        for i in range(2):
                            nc.tensor.matmul(
                                pq[:, i * 512 : (i + 1) * 512],
                                w_sb[:],
                                x2[:, h * 1024 + i * 512 : h * 1024 + (i + 1) * 512],
                                start=True,
                                stop=True,
                            )
                        nc.vector.tensor_copy(
                            out=dst[:, h * 1024 : (h + 1) * 1024], in_=pq[:]
                        )

                va = persist.tile([128, 2, S // KT, 65], F32R, tag=f"v{p}")
                nc.sync.dma_start(va[:, :, :, 64:65], vones[:])  # ones column
                for g8 in range(2):  # groups of 8 s-tiles
                    pv = ps_s.tile([128, 1024], F32, tag="s_ps")
                    for i in range(8):
                        st = g8 * 8 + i
                        nc.tensor.matmul(
                            pv[:, i * 128 : (i + 1) * 128],
                            x2[:, st * 128 : (st + 1) * 128],
                            wv_sb[:],
                            start=True,
                            stop=True,
                        )
                    for i in range(8):
                        st = g8 * 8 + i
                        nc.vector.tensor_copy(
                            out=va[:, :, st, 0:64],
                            in_=pv[:, i * 128 : (i + 1) * 128].rearrange(
                                "p (two c) -> p two c", two=2
                            ),
                        )
                qT.append(q_sb)
                kT.append(k_sb)
                vaug.append(va)

            # ---- phase 2: attention + output projection, per q-block ----
            for qb in range(NQB):
                q0 = qb * QB
                nkt = (q0 + QB) // KT  # causal k-tiles
                cstrip = strip.tile([128, PAIRS, QB], F32R, tag="cstrip")
                sums8 = small.tile([HPC, QB], F32, tag="sums8")
                for head in range(HPC):
                    p, hh = head // 2, head % 2
                    r0 = hh * 64
                    cps = ps_c.tile([65, QB], F32, tag="c_ps")
                    # all scores groups first (PE streams without ACT stalls),
                    # then all ctx matmuls (their exps completed meanwhile)
                    e_tiles = []
                    for g0 in range(0, nkt, GROUP):
                        g1 = min(g0 + GROUP, nkt)
                        ncols = (g1 - g0) * QB
                        sps = ps_s.tile([128, 1024], F32, tag="s_ps")
                        for j in range(g0, g1):
                            jj = j - g0
                            nc.tensor.matmul(
                                sps[:, jj * QB : (jj + 1) * QB],
                                kT[p][r0 : r0 + 64, j * KT : (j + 1) * KT],
                                qT[p][r0 : r0 + 64, q0 : q0 + QB],
                                start=True,
                                stop=True,
                            )
                            t = j - (nkt - DIAG)
                            if t >= 0:  # diagonal tile: additive causal mask
                                nc.vector.tensor_add(
                                    sps[:, jj * QB : (jj + 1) * QB],
                                    sps[:, jj * QB : (jj + 1) * QB],
                                    mask_sb[:, t, :],
                                )
                        e_sb = epool.tile([128, 1024], F32R, tag="e")
                        nc.scalar.activation(
                            out=e_sb[:, 0:ncols],
                            in_=sps[:, 0:ncols],
                            func=EXPF,
                            scale=0.125,
                        )
                        e_tiles.append((g0, g1, e_sb))
                    for g0, g1, e_sb in e_tiles:
                        for j in range(g0, g1):
                            jj = j - g0
                            nc.tensor.matmul(
                                cps[:],
                                vaug[p][:, hh, j, :],
                                e_sb[:, jj * QB : (jj + 1) * QB],
                                start=(j == 0),
                                stop=(j == nkt - 1),
                            )
                    # stage ctx + denominator out of PSUM; normalize later
                    nc.vector.tensor_copy(
                        cstrip[r0 : r0 + 64, p, :], cps[0:64, :]
                    )
                    srow = small.tile([65, QB], F32, tag="srow")
                    nc.vector.tensor_copy(srow[64:65, :], cps[64:65, :])
                    nc.sync.dma_start(sums8[head : head + 1, :], srow[64:65, :])
                # one batched reciprocal for all heads, then per-head bcast+mul
                rec8 = small.tile([HPC, QB], F32, tag="rec8")
                nc.vector.reciprocal(rec8[:], sums8[:])
                for head in range(HPC):
                    p, hh = head // 2, head % 2
                    r0 = hh * 64
                    rtmp = small.tile([1, QB], F32, tag="rtmp")
                    nc.sync.dma_start(rtmp[:], rec8[head : head + 1, :])
                    bcast = small.tile([128, QB], F32, tag="bcast")
                    nc.gpsimd.partition_broadcast(bcast[:], rtmp[:], channels=128)
                    nc.vector.tensor_mul(
                        cstrip[r0 : r0 + 64, p, :],
                        cstrip[r0 : r0 + 64, p, :],
                        bcast[r0 : r0 + 64, :],
                    )

                for st in range(QB // 128):
                    s0 = q0 + st * 128
                    for oc in range(V // 512):
                        ops = ps_o.tile([128, 512], F32, tag="o_ps")
                        for p in range(PAIRS):
                            nc.tensor.matmul(
                                ops[:],
                                cstrip[:, p, st * 128 : (st + 1) * 128],
                                woT_sb[:, p, oc * 512 : (oc + 1) * 512],
                                start=(p == 0),
                                stop=(p == PAIRS - 1),
                            )
                        o_sb = outsb.tile([128, 512], F32R, tag="osb")
                        nc.vector.tensor_copy(o_sb[:], ops[:])
                        nc.sync.dma_start(
                            out[s0 : s0 + 128, oc * 512 : (oc + 1) * 512], o_sb[:]
                        )
    nc.compile()
    return nc


_NC = None


def _get_nc():
    global _NC
    if _NC is None:
        _NC = _build_nc()
    return _NC


def _host_inputs(x, w_qkv, w_out):
    """Build the 8 per-core input maps from the full tensors."""
    x = np.asarray(x, dtype=np.float32)
    w_qkv = np.asarray(w_qkv, dtype=np.float32)
    w_out = np.asarray(w_out, dtype=np.float32)

    def blockdiag(m):  # m [64, 64] -> [128, 128]
        z = np.zeros((128, 128), dtype=np.float32)
        z[0:64, 0:64] = m
        z[64:128, 64:128] = m
        return z

    wq = blockdiag(w_qkv[0:64].T.copy())
    wk = blockdiag(w_qkv[64:128].T.copy())
    wv = blockdiag(w_qkv[128:192].T.copy())

    kk = np.arange(128)[:, None]
    qq = np.arange(QB)[None, :]
    maskadd = np.stack(
        [np.where(kk <= qq - 128 * t, 0.0, NEG) for t in range(DIAG)]
    ).astype(np.float32)

    vones = np.ones((128, 2, S // KT, 1), dtype=np.float32)
    in_maps = []
    for c in range(8):
        b, g = c // 2, c % 2
        cols = slice(g * 512, (g + 1) * 512)
        in_maps.append(
            {
                "xT": np.ascontiguousarray(x[b][:, cols].T),
                "wq": wq,
                "wk": wk,
                "wv": wv,
                "woT": np.ascontiguousarray(w_out[:, cols].T),
                "maskadd": maskadd,
                "vones": vones,
            }
        )
    return in_maps


def run(x, w_qkv, w_out, trace=False, tmpdir=None):
    from concourse.bass_utils import run_bass_kernel_spmd

    nc = _get_nc()
    in_maps = _host_inputs(x, w_qkv, w_out)
    res = run_bass_kernel_spmd(
        nc, in_maps, core_ids=list(range(8)), trace=trace, tmpdir=tmpdir
    )
    outs = [r["out"] for r in res.results]
    full = np.empty((B, S, V), dtype=np.float32)
    for b in range(B):
        full[b] = outs[2 * b] + outs[2 * b + 1]
    return full, res


def kernel(x, w_qkv, w_out):
    full, _ = run(x, w_qkv, w_out)
    return full

